# revision 6
# baseline (speedup 1.0000x reference)
"""GATv2Layer (nn_GATv2Layer_42356967473536) — Trainium2 Bass kernel.

Math
----
The reference computes
    hp   = einsum('bnf,hfd->bhnd', h, W)          # per-head projections
    e    = leaky_relu(hp @ hp^T)
    attn = softmax(e, axis=-1)
    out  = hp * sum(attn, axis=-1, keepdims=True) # row-sums of softmax == 1
    out  = concat_heads(out)                      # (B, N, H*D)
    res  = alpha * out + (1 - alpha) * h

sum(softmax(x), -1) is identically 1, so the whole attention block is a
no-op and, with F == H*D == 256, the layer collapses to one matmul per
batch element:
    res_b = h_b @ M,   M = alpha * Wc + (1 - alpha) * I_256,
    Wc[f, hd] = W[hd // 64, f, hd % 64]

Precision: the harness gate is Frobenius rel err < 2e-2.  bf16 inputs +
bf16 output keep the error ~3e-3 (fp32 PSUM accumulation), while halving
DMA traffic and quadrupling PE throughput vs fp32 (fp32 matmul = 2
emitted passes x 2 cycles/col).

Sharding
--------
Data-parallel over batch B=8 -> one batch element per NeuronCore.
Per core: outT_b = M^T @ h_b^T as (128f x 128d) @ (128f x Nn) PE
matmuls accumulating over the two 128-row halves of F.  The host passes
[M | h_b^T] concatenated in bf16 (contraction dim must sit on SBUF
partitions) and transposes the (256, 2048) bf16 per-core result back on
gather.

Kernel structure (raw bass Block, hand-rolled semaphores)
---------------------------------------------------------
- loads:  3 column-spans x 2 F-halves on the two HWDGE rings (sync +
  scalar); the first span is small (M + first node chunk) to minimize
  time-to-first-matmul.
- PE:     zero-matmul warmups (256-wide) keep PE busy from block start
  so the HAM activity window ramps the clock 1.2->2.4 GHz as early as
  possible; then 10 bf16 accumulation groups (node chunks
  256,512,512,512,256 x 2 d-halves).  Groups 8,9 recycle PSUM banks
  0,1 behind a copy-semaphore guard.
- copies: PSUM -> SBUF bf16 downcast, split DVE (chunks 0,2,4) / ACT
  (chunks 1,3) so the combined copy pace beats the warm PE pace.
- stores: bf16 per chunk-range on alternating rings; the last chunk is
  small (256 nodes) and split by d-half across both rings to shorten
  the completion tail.
"""

import os
import sys
import types
from contextlib import ExitStack

import numpy as np

B, N, F = 8, 2048, 256
H, D = 4, 64
P = 128
KO = 2                 # contraction subtiles (F = 2 * 128)
NCORES = 8
W_ALL = F + N          # hm input: [M | hT] = 2304 columns
NWARM = 12
WU_W = 256             # warmup matmul width

# load column-spans of hm, per ko-half, one DMA each per ring
SPANS = [(0, 512), (512, 1024), (1024, 2304)]
# matmul node chunks: (width, load-span index that covers it)
CHUNKS = [(256, 0), (512, 1), (512, 2), (512, 2), (256, 2)]

_NC = None
LAST_EXEC_TIME_NS = None
LAST_TRACE_PATH = None


def _ensure_axon_ntff_hook():
    """Make run_bass_kernel_spmd(trace=True) work under axon in this image
    (antenv.axon_hooks is absent; trn_boot carries the ctypes impl)."""
    try:
        import antenv.axon_hooks  # noqa: F401
        return
    except ImportError:
        pass
    try:
        from trn_agent_boot.trn_boot import _ntff_profile_via_ctypes

        hook = _ntff_profile_via_ctypes("/opt/axon/libaxon_pjrt.so")
        mod = types.ModuleType("antenv.axon_hooks")
        mod.get_axon_ntff_profile_hook = lambda: hook
        mod.set_axon_ntff_profile_hook = lambda h: None
        sys.modules["antenv.axon_hooks"] = mod
        import concourse.bass_utils as bass_utils

        bass_utils.upload_artifacts = lambda tmpdir: tmpdir  # no S3 here
    except Exception:
        pass


def _build_nc():
    from concourse import bacc, mybir

    f32 = mybir.dt.float32
    bf16 = mybir.dt.bfloat16

    nc = bacc.Bacc()
    hm = nc.declare_dram_parameter("hm", [F, W_ALL], bf16, isOutput=False)
    outT = nc.declare_dram_parameter("outT", [F, N], bf16, isOutput=True)

    hm_r = hm.rearrange("(ko p) n -> p ko n", p=P)     # (128, 2, 2304)
    oT_r = outT.rearrange("(dh p) n -> p dh n", p=P)   # (128, 2, 2048)

    # chunk g -> (node0, width, dh) per psum group, DVE vs ACT ownership
    groups = []
    node = 0
    for ci, (w, si) in enumerate(CHUNKS):
        for dh in range(KO):
            groups.append((ci, node, w, dh, si))
        node += w
    DVE_CHUNKS = (0, 2, 4)
    ACT_CHUNKS = (1, 3)

    with ExitStack() as es:
        h_sb = es.enter_context(nc.sbuf_tensor("h_sb", [P, KO, W_ALL], bf16))
        o_sb = es.enter_context(nc.sbuf_tensor("o_sb", [P, KO, N], bf16))
        wu_sb = es.enter_context(nc.sbuf_tensor("wu_sb", [P, WU_W], bf16))
        psum = [
            es.enter_context(nc.psum_tensor(f"psum{i}", [P, 512], f32))
            for i in range(8)
        ]
        sp_sems = [
            es.enter_context(nc.semaphore(f"sp_sem{s}")) for s in range(len(SPANS))
        ]
        wu_sem = es.enter_context(nc.semaphore("wu_sem"))
        mm_sem = es.enter_context(nc.semaphore("mm_sem"))
        cv_sem = es.enter_context(nc.semaphore("cv_sem"))  # DVE copies
        ca_sem = es.enter_context(nc.semaphore("ca_sem"))  # ACT copies
        st_sem = es.enter_context(nc.semaphore("st_sem"))
        blk = es.enter_context(nc.Block())

        @blk.sync
        def _(sync):
            for si, (a, b) in enumerate(SPANS):  # ko=0 halves
                sync.dma_start(h_sb[:, 0, a:b], hm_r[:, 0, a:b]).then_inc(
                    sp_sems[si], 16
                )
            # SA: chunks 0+1 (cols 0:768)
            sync.wait_ge(cv_sem, 2)
            sync.wait_ge(ca_sem, 2)
            sync.dma_start(oT_r[:, :, 0:768], o_sb[:, :, 0:768]).then_inc(
                st_sem, 16
            )
            # SC: chunk 3 (cols 1280:1792)
            sync.wait_ge(ca_sem, 4)
            sync.dma_start(oT_r[:, :, 1280:1792], o_sb[:, :, 1280:1792]).then_inc(
                st_sem, 16
            )
            # SD1: chunk 4, dh=1
            sync.wait_ge(cv_sem, 6)
            sync.dma_start(
                oT_r[:, 1, 1792:2048], o_sb[:, 1, 1792:2048]
            ).then_inc(st_sem, 16)
            sync.wait_ge(st_sem, 80)  # all stores landed before kernel exit

        @blk.scalar
        def _(scalar):
            for si, (a, b) in enumerate(SPANS):  # ko=1 halves
                scalar.dma_start(h_sb[:, 1, a:b], hm_r[:, 1, a:b]).then_inc(
                    sp_sems[si], 16
                )
            # ACT copies: chunks 1, 3 interleaved with its stores
            for ci in ACT_CHUNKS:
                for g, (gc, node, w, dh, si) in enumerate(groups):
                    if gc != ci:
                        continue
                    nc.scalar.copy(
                        o_sb[:, dh, node:node + w], psum[g % 8][:, :w]
                    )._wait_ge(mm_sem, g + 1).then_inc(ca_sem, 1)
                if ci == 1:
                    # SB: chunk 2 (cols 768:1280), DVE-copied
                    scalar.wait_ge(cv_sem, 4)
                    scalar.dma_start(
                        oT_r[:, :, 768:1280], o_sb[:, :, 768:1280]
                    ).then_inc(st_sem, 16)
            # SD0: chunk 4, dh=0
            scalar.wait_ge(cv_sem, 5)
            scalar.dma_start(
                oT_r[:, 0, 1792:2048], o_sb[:, 0, 1792:2048]
            ).then_inc(st_sem, 16)

        @blk.gpsimd
        def _(gpsimd):
            nc.gpsimd.memset(wu_sb[:], 0.0).then_inc(wu_sem, 1)

        @blk.vector
        def _(vector):
            # DVE copies: chunks 0, 2, 4
            for ci in DVE_CHUNKS:
                for g, (gc, node, w, dh, si) in enumerate(groups):
                    if gc != ci:
                        continue
                    nc.vector.tensor_copy(
                        o_sb[:, dh, node:node + w], psum[g % 8][:, :w]
                    )._wait_ge(mm_sem, g + 1).then_inc(cv_sem, 1)

        @blk.tensor
        def _(tensor):
            tensor.wait_ge(wu_sem, 1)
            for _ in range(NWARM):  # HAM warm-up on zeros
                nc.tensor.matmul(
                    psum[0][:, :WU_W], wu_sb[:, :P], wu_sb[:], start=True, stop=True
                )
            for g, (ci, node, w, dh, si) in enumerate(groups):
                if dh == 0:
                    tensor.wait_ge(sp_sems[si], 32)  # both ko halves of span
                if g == 8:
                    tensor.wait_ge(cv_sem, 2)  # banks 0,1 recycled for chunk 4
                b = g % 8
                col = F + node
                nc.tensor.matmul(
                    psum[b][:, :w],
                    h_sb[:, 0, dh * P:(dh + 1) * P],
                    h_sb[:, 0, col:col + w],
                    start=True,
                    stop=False,
                )
                nc.tensor.matmul(
                    psum[b][:, :w],
                    h_sb[:, 1, dh * P:(dh + 1) * P],
                    h_sb[:, 1, col:col + w],
                    start=False,
                    stop=True,
                ).then_inc(mm_sem, 1)

    nc.finalize()
    return nc


def kernel(h, adj, W, alpha_res):
    global _NC, LAST_EXEC_TIME_NS, LAST_TRACE_PATH

    import ml_dtypes

    bf16 = ml_dtypes.bfloat16

    h = np.asarray(h, dtype=np.float32)
    W = np.asarray(W, dtype=np.float32)
    alpha = float(np.asarray(alpha_res))
    # adj is unused by the reference's math.

    # M = alpha * concat-heads(W) + (1 - alpha) * I  (residual folded in)
    Wc = W.transpose(1, 0, 2).reshape(F, F)
    Mmat = (alpha * Wc + (1.0 - alpha) * np.eye(F, dtype=np.float32)).astype(
        np.float32
    )

    trace = os.environ.get("BASS_TRACE", "").lower() in ("1", "true", "yes")
    if trace:
        _ensure_axon_ntff_hook()

    from concourse.bass_utils import run_bass_kernel_spmd

    if _NC is None:
        _NC = _build_nc()

    in_maps = [
        {
            "hm": np.ascontiguousarray(
                np.concatenate([Mmat, h[b].T], axis=1)
            ).astype(bf16)
        }
        for b in range(NCORES)
    ]
    res = run_bass_kernel_spmd(
        _NC, in_maps, core_ids=list(range(NCORES)), trace=trace
    )
    LAST_EXEC_TIME_NS = res.exec_time_ns
    if res.instructions_and_trace is not None:
        LAST_TRACE_PATH = res.instructions_and_trace[1]

    return np.ascontiguousarray(
        np.stack(
            [res.results[b]["outT"].astype(np.float32).T for b in range(NCORES)]
        )
    )


# revision 9
# speedup vs baseline: 1.0449x; 1.0449x over previous
"""GATv2Layer (nn_GATv2Layer_42356967473536) — Trainium2 Bass kernel.

Math
----
The reference computes
    hp   = einsum('bnf,hfd->bhnd', h, W)          # per-head projections
    e    = leaky_relu(hp @ hp^T)
    attn = softmax(e, axis=-1)
    out  = hp * sum(attn, axis=-1, keepdims=True) # row-sums of softmax == 1
    out  = concat_heads(out)                      # (B, N, H*D)
    res  = alpha * out + (1 - alpha) * h

sum(softmax(x), -1) is identically 1, so the whole attention block is a
no-op and, with F == H*D == 256, the layer collapses to one matmul per
batch element:
    res_b = h_b @ M,   M = alpha * Wc + (1 - alpha) * I_256,
    Wc[f, hd] = W[hd // 64, f, hd % 64]

Precision: the harness gate is Frobenius rel err < 2e-2.  bf16 inputs +
bf16 output keep the error ~3e-3 (fp32 PSUM accumulation), while halving
DMA traffic and quadrupling PE throughput vs fp32 (fp32 matmul = 2
emitted passes x 2 cycles/col).

Sharding
--------
Data-parallel over batch B=8 -> one batch element per NeuronCore.
Per core: outT_b = M^T @ h_b^T as (128f x 128d) @ (128f x Nn) PE
matmuls accumulating over the two 128-row halves of F.  The host passes
[M | h_b^T] concatenated in bf16 (contraction dim must sit on SBUF
partitions) and transposes the (256, 2048) bf16 per-core result back on
gather.

Kernel structure (raw bass Block, hand-rolled semaphores)
---------------------------------------------------------
- loads:  3 column-spans x 2 F-halves on the two HWDGE rings (sync +
  scalar); the first span is small (M + first node chunk) to minimize
  time-to-first-matmul.
- PE:     zero-matmul warmups (256-wide) keep PE busy from block start
  so the HAM activity window ramps the clock 1.2->2.4 GHz as early as
  possible; then 10 bf16 accumulation groups (node chunks
  256,512,512,512,256 x 2 d-halves).  Groups 8,9 recycle PSUM banks
  0,1 behind a copy-semaphore guard.
- copies: PSUM -> SBUF bf16 downcast, split DVE (chunks 0,2,4) / ACT
  (chunks 1,3) so the combined copy pace beats the warm PE pace.
- stores: bf16 per chunk-range on alternating rings; the last chunk is
  small (256 nodes) and split by d-half across both rings to shorten
  the completion tail.
"""

import os
import sys
import types
from contextlib import ExitStack

import numpy as np

B, N, F = 8, 2048, 256
H, D = 4, 64
P = 128
KO = 2                 # contraction subtiles (F = 2 * 128)
NCORES = 8
W_ALL = F + N          # hm input: [M | hT] = 2304 columns
NWARM = 14
WU_W = 256             # warmup matmul width

# load column-spans of hm (both ko halves in one DMA), one span per DMA
# queue (sync=SP-HWDGE, scalar=ACT-HWDGE, gpsimd=Pool-SWDGE) so the three
# completion-semaphore chains run in parallel instead of serialized.
SPANS = [(0, 768), (768, 1536), (1536, 2304)]
# matmul node chunks: (width, load-span index that covers it)
CHUNKS = [(512, 0), (512, 1), (256, 1), (512, 2), (256, 2)]

_NC = None
LAST_EXEC_TIME_NS = None
LAST_TRACE_PATH = None


def _ensure_axon_ntff_hook():
    """Make run_bass_kernel_spmd(trace=True) work under axon in this image
    (antenv.axon_hooks is absent; trn_boot carries the ctypes impl)."""
    try:
        import antenv.axon_hooks  # noqa: F401
        return
    except ImportError:
        pass
    try:
        from trn_agent_boot.trn_boot import _ntff_profile_via_ctypes

        hook = _ntff_profile_via_ctypes("/opt/axon/libaxon_pjrt.so")
        mod = types.ModuleType("antenv.axon_hooks")
        mod.get_axon_ntff_profile_hook = lambda: hook
        mod.set_axon_ntff_profile_hook = lambda h: None
        sys.modules["antenv.axon_hooks"] = mod
        import concourse.bass_utils as bass_utils

        bass_utils.upload_artifacts = lambda tmpdir: tmpdir  # no S3 here
    except Exception:
        pass


def _build_nc():
    from concourse import bacc, mybir

    f32 = mybir.dt.float32
    bf16 = mybir.dt.bfloat16

    nc = bacc.Bacc()
    hm = nc.declare_dram_parameter("hm", [F, W_ALL], bf16, isOutput=False)
    outT = nc.declare_dram_parameter("outT", [F, N], bf16, isOutput=True)

    hm_r = hm.rearrange("(ko p) n -> p ko n", p=P)     # (128, 2, 2304)
    oT_r = outT.rearrange("(dh p) n -> p dh n", p=P)   # (128, 2, 2048)

    # chunk g -> (node0, width, dh) per psum group, DVE vs ACT ownership
    groups = []
    node = 0
    for ci, (w, si) in enumerate(CHUNKS):
        for dh in range(KO):
            groups.append((ci, node, w, dh, si))
        node += w
    DVE_CHUNKS = (0, 2, 4)
    ACT_CHUNKS = (1, 3)

    with ExitStack() as es:
        h_sb = es.enter_context(nc.sbuf_tensor("h_sb", [P, KO, W_ALL], bf16))
        o_sb = es.enter_context(nc.sbuf_tensor("o_sb", [P, KO, N], bf16))
        wu_sb = es.enter_context(nc.sbuf_tensor("wu_sb", [P, WU_W], bf16))
        psum = [
            es.enter_context(nc.psum_tensor(f"psum{i}", [P, 512], f32))
            for i in range(8)
        ]
        sp_sems = [
            es.enter_context(nc.semaphore(f"sp_sem{s}")) for s in range(len(SPANS))
        ]
        wu_sem = es.enter_context(nc.semaphore("wu_sem"))
        mm_sem = es.enter_context(nc.semaphore("mm_sem"))
        cv_sem = es.enter_context(nc.semaphore("cv_sem"))  # DVE copies
        ca_sem = es.enter_context(nc.semaphore("ca_sem"))  # ACT copies
        st_sem = es.enter_context(nc.semaphore("st_sem"))
        blk = es.enter_context(nc.Block())

        @blk.sync
        def _(sync):
            a, b = SPANS[0]
            sync.dma_start(h_sb[:, :, a:b], hm_r[:, :, a:b]).then_inc(
                sp_sems[0], 16
            )
            # SA: chunk 0 (nodes 0:512)
            sync.wait_ge(cv_sem, 2)
            sync.dma_start(oT_r[:, :, 0:512], o_sb[:, :, 0:512]).then_inc(
                st_sem, 16
            )
            # SC: chunk 3 (nodes 1280:1792)
            sync.wait_ge(ca_sem, 4)
            sync.dma_start(oT_r[:, :, 1280:1792], o_sb[:, :, 1280:1792]).then_inc(
                st_sem, 16
            )
            # SD1: chunk 4, dh=1 (nodes 1792:2048)
            sync.wait_ge(cv_sem, 6)
            sync.dma_start(
                oT_r[:, 1, 1792:2048], o_sb[:, 1, 1792:2048]
            ).then_inc(st_sem, 16)
            sync.wait_ge(st_sem, 80)  # all stores landed before kernel exit

        @blk.scalar
        def _(scalar):
            a, b = SPANS[1]
            scalar.dma_start(h_sb[:, :, a:b], hm_r[:, :, a:b]).then_inc(
                sp_sems[1], 16
            )
            # ACT copies: chunks 1, 3 interleaved with its stores
            for ci in ACT_CHUNKS:
                for g, (gc, node, w, dh, si) in enumerate(groups):
                    if gc != ci:
                        continue
                    nc.scalar.copy(
                        o_sb[:, dh, node:node + w], psum[g % 8][:, :w]
                    )._wait_ge(mm_sem, g + 1).then_inc(ca_sem, 1)
                if ci == 1:
                    # SB: chunks 1+2 (nodes 512:1280); c1 is ACT-copied
                    # (already done here), c2 is DVE-copied.
                    scalar.wait_ge(cv_sem, 4)
                    scalar.dma_start(
                        oT_r[:, :, 512:1280], o_sb[:, :, 512:1280]
                    ).then_inc(st_sem, 16)
            # SD0: chunk 4, dh=0
            scalar.wait_ge(cv_sem, 5)
            scalar.dma_start(
                oT_r[:, 0, 1792:2048], o_sb[:, 0, 1792:2048]
            ).then_inc(st_sem, 16)

        @blk.gpsimd
        def _(gpsimd):
            nc.gpsimd.memset(wu_sb[:], 0.0).then_inc(wu_sem, 1)
            a, b = SPANS[2]
            gpsimd.dma_start(h_sb[:, :, a:b], hm_r[:, :, a:b]).then_inc(
                sp_sems[2], 16
            )

        @blk.vector
        def _(vector):
            # DVE copies: chunks 0, 2, 4
            for ci in DVE_CHUNKS:
                for g, (gc, node, w, dh, si) in enumerate(groups):
                    if gc != ci:
                        continue
                    nc.vector.tensor_copy(
                        o_sb[:, dh, node:node + w], psum[g % 8][:, :w]
                    )._wait_ge(mm_sem, g + 1).then_inc(cv_sem, 1)

        @blk.tensor
        def _(tensor):
            tensor.wait_ge(wu_sem, 1)
            for _ in range(NWARM):  # HAM warm-up on zeros
                nc.tensor.matmul(
                    psum[0][:, :WU_W], wu_sb[:, :P], wu_sb[:], start=True, stop=True
                )
            for g, (ci, node, w, dh, si) in enumerate(groups):
                if dh == 0:
                    tensor.wait_ge(sp_sems[si], 16)  # span landed (both ko)
                if g == 8:
                    tensor.wait_ge(cv_sem, 2)  # banks 0,1 recycled for chunk 4
                b = g % 8
                col = F + node
                nc.tensor.matmul(
                    psum[b][:, :w],
                    h_sb[:, 0, dh * P:(dh + 1) * P],
                    h_sb[:, 0, col:col + w],
                    start=True,
                    stop=False,
                )
                nc.tensor.matmul(
                    psum[b][:, :w],
                    h_sb[:, 1, dh * P:(dh + 1) * P],
                    h_sb[:, 1, col:col + w],
                    start=False,
                    stop=True,
                ).then_inc(mm_sem, 1)

    nc.finalize()
    return nc


def kernel(h, adj, W, alpha_res):
    global _NC, LAST_EXEC_TIME_NS, LAST_TRACE_PATH

    import ml_dtypes

    bf16 = ml_dtypes.bfloat16

    h = np.asarray(h, dtype=np.float32)
    W = np.asarray(W, dtype=np.float32)
    alpha = float(np.asarray(alpha_res))
    # adj is unused by the reference's math.

    # M = alpha * concat-heads(W) + (1 - alpha) * I  (residual folded in)
    Wc = W.transpose(1, 0, 2).reshape(F, F)
    Mmat = (alpha * Wc + (1.0 - alpha) * np.eye(F, dtype=np.float32)).astype(
        np.float32
    )

    trace = os.environ.get("BASS_TRACE", "").lower() in ("1", "true", "yes")
    if trace:
        _ensure_axon_ntff_hook()

    from concourse.bass_utils import run_bass_kernel_spmd

    if _NC is None:
        _NC = _build_nc()

    in_maps = [
        {
            "hm": np.ascontiguousarray(
                np.concatenate([Mmat, h[b].T], axis=1)
            ).astype(bf16)
        }
        for b in range(NCORES)
    ]
    res = run_bass_kernel_spmd(
        _NC, in_maps, core_ids=list(range(NCORES)), trace=trace
    )
    LAST_EXEC_TIME_NS = res.exec_time_ns
    if res.instructions_and_trace is not None:
        LAST_TRACE_PATH = res.instructions_and_trace[1]

    return np.ascontiguousarray(
        np.stack(
            [res.results[b]["outT"].astype(np.float32).T for b in range(NCORES)]
        )
    )
